# revision 2
# baseline (speedup 1.0000x reference)
"""Trainium2 kernel for out = A @ W2 @ B.T with banded Gaussian W2.

Math: W2 = W1*W1 where W1[i,j] = exp(-(i-j)^2/(2*8^2)) truncated below 1e-10.
W1 > eps only for |i-j| <= 54, so in 128-blocks W2 is block-tridiagonal AND
translation-invariant: only three distinct 128x128 blocks exist (diag D0,
super-diag U = W2[j-1,j], sub-diag L = W2[j+1,j] = U.T).

Strategy (data-parallel over A's rows, 8 cores, no collectives):
  - host: transpose A and B once, quantize to bf16, build the three W2
    blocks; B.T is retiled so every device DMA is fully contiguous.
  - each core gets A.T slab [4096, 1024], full B.T (tiled), the W2 pack.
  - phase 1 (once): TT = W2 @ A.T  (= (A_slab @ W2).T), banded block-tridiag
    matmuls over the narrow A-slab; TT [4096, 1024] bf16 stays in SBUF.
  - phase 2 (per 512-col chunk nu): out[:, nu] = TT.T @ B.T[:, nu], with all
    8 PSUM banks accumulating the 8 m-tiles while B.T streams through once.
  - bf16 matmuls run at 1 cyc/row (same as fp32r) but halve DMA traffic and
    enable fast weight load; rel err ~2e-3 is far inside the 2e-2 gate.
"""

import numpy as np
import ml_dtypes

import concourse.bass as bass
import concourse.mybir as mybir
from concourse import bacc
from concourse.bass_utils import run_bass_kernel_spmd
from concourse.tile import TileContext

P = 128          # partition / block size
N = 4096         # inner dims (A cols, B rows/cols)
M_FULL = 8192    # A rows
NCORES = 8
MS = M_FULL // NCORES   # 1024 rows of A per core
NK = N // P      # 32 contraction blocks
NM = MS // P     # 8 m-tiles per core
CW = 512         # output column chunk width (= 1 PSUM bank of fp32)
NCH = N // CW    # 8 chunks
NH = MS // CW    # 2 column-halves of the A.T slab in phase 1

SIGMA = 8.0
TRUNC_EPS = 1e-10

BF16 = np.dtype(ml_dtypes.bfloat16)

_COMPILED = {}


def _w2_block(dist):
    """W2 entries for a matrix of absolute diagonal distances."""
    d = dist.astype(np.float32)
    w1 = np.exp(-(d * d) / np.float32(2.0 * SIGMA * SIGMA)).astype(np.float32)
    w1 = np.where(w1 > np.float32(TRUNC_EPS), w1, np.float32(0.0)).astype(np.float32)
    return (w1 * w1).astype(np.float32)


def _build_w2_pack():
    a = np.arange(P)[:, None]
    b = np.arange(P)[None, :]
    d0 = _w2_block(np.abs(a - b))          # W2[j, j]
    u = _w2_block(np.abs(a - b - P))       # W2[j-1, j]
    l = _w2_block(np.abs(P + a - b))       # W2[j+1, j]
    pack = np.concatenate([d0, u, l], axis=1)  # [128, 384]
    return np.ascontiguousarray(pack.astype(BF16))


def _build_program(reps=1):
    """Build + compile the Bass program (one NEFF, run SPMD on 8 cores)."""
    nc = bacc.Bacc("TRN2", target_bir_lowering=False, debug=False)
    f32 = mybir.dt.float32
    bf16 = mybir.dt.bfloat16

    at_dram = nc.dram_tensor("at", [N, MS], bf16, kind="ExternalInput").ap()
    # B.T retiled on host: bt[k, nu, p, f] = B.T[k*128+p, nu*512+f]
    bt_dram = nc.dram_tensor(
        "bt", [NK, NCH, P, CW], bf16, kind="ExternalInput"
    ).ap()
    w2_dram = nc.dram_tensor("w2", [P, 3 * P], bf16, kind="ExternalInput").ap()
    # out tiled: out[m, nu, p, f] = out_slab[m*128+p, nu*512+f]
    out_dram = nc.dram_tensor(
        "out", [NM, NCH, P, CW], f32, kind="ExternalOutput"
    ).ap()

    with TileContext(nc) as tc:
        with (
            tc.tile_pool(name="const", bufs=1) as const_pool,
            tc.tile_pool(name="atp", bufs=6) as at_pool,
            tc.tile_pool(name="ttp", bufs=1) as tt_pool,
            tc.tile_pool(name="btp", bufs=8) as bt_pool,
            tc.tile_pool(name="obp", bufs=6) as ob_pool,
            tc.tile_pool(name="psp", bufs=8, space="PSUM") as ps_pool,
        ):
            w2_sb = const_pool.tile([P, 3 * P], bf16, tag="w2", name="w2_sb")
            nc.sync.dma_start(w2_sb, w2_dram)
            # lhsT for contribution d: W2[j+d, j]
            w2_lhsT = {
                0: w2_sb[:, 0:P],
                -1: w2_sb[:, P:2 * P],
                1: w2_sb[:, 2 * P:3 * P],
            }

            for rep in range(reps):
                # --- phase 1: TT = W2 @ A.T ([4096, 1024] bf16, in SBUF)
                at_tiles = [None] * NK

                def get_at(k, rep=rep):
                    if at_tiles[k] is None:
                        at_t = at_pool.tile([P, MS], bf16, tag="at",
                                            name=f"at_sb_{rep}_{k}")
                        nc.sync.dma_start(at_t, at_dram[k * P:(k + 1) * P, :])
                        at_tiles[k] = at_t
                    return at_tiles[k]

                tt_tiles = []
                for j in range(NK):
                    tt_t = tt_pool.tile([P, MS], bf16, tag=f"tt{j}",
                                        name=f"tt_sb_{rep}_{j}")
                    dlist = [d for d in (-1, 0, 1) if 0 <= j + d < NK]
                    for h in range(NH):
                        hs = bass.ts(h, CW)
                        ps_t = ps_pool.tile([P, CW], f32, tag="ps",
                                            name=f"ps_t_{rep}_{j}_{h}")
                        for i, d in enumerate(dlist):
                            nc.tensor.matmul(
                                ps_t,
                                lhsT=w2_lhsT[d],
                                rhs=get_at(j + d)[:, hs],
                                start=(i == 0),
                                stop=(i == len(dlist) - 1),
                            )
                        nc.vector.tensor_copy(tt_t[:, hs], ps_t)
                    tt_tiles.append(tt_t)

                # --- phase 2: out = TT.T @ B.T, streamed in 512-col chunks
                for nu in range(NCH):
                    ps_o = [
                        ps_pool.tile([P, CW], f32, tag="ps",
                                     name=f"ps_o_{rep}_{nu}_{m}")
                        for m in range(NM)
                    ]
                    for k in range(NK):
                        bt_t = bt_pool.tile([P, CW], bf16, tag="bt",
                                            name=f"bt_sb_{rep}_{nu}_{k}")
                        nc.sync.dma_start(bt_t, bt_dram[k, nu])
                        for m in range(NM):
                            nc.tensor.matmul(
                                ps_o[m],
                                lhsT=tt_tiles[k][:, m * P:(m + 1) * P],
                                rhs=bt_t,
                                start=(k == 0),
                                stop=(k == NK - 1),
                            )
                    for m in range(NM):
                        ob_t = ob_pool.tile([P, CW], f32, tag="ob",
                                            name=f"ob_sb_{rep}_{nu}_{m}")
                        if m % 2 == 0:
                            nc.vector.tensor_copy(ob_t, ps_o[m])
                        else:
                            nc.scalar.copy(ob_t, ps_o[m])
                        nc.sync.dma_start(out_dram[m, nu], ob_t)

    nc.compile()
    return nc


def _get_program():
    if "nc" not in _COMPILED:
        _COMPILED["nc"] = _build_program()
    return _COMPILED["nc"]


def _prep_inputs(A, B):
    """Host-side shard + quantize + retile. Returns per-core input maps."""
    a_t = np.ascontiguousarray(A.T).astype(BF16)          # [4096, 8192]
    bt = np.ascontiguousarray(B.T).astype(BF16)           # [4096, 4096]
    # bt[k, nu, p, f] = B.T[k*128+p, nu*512+f], contiguous per [128,512] tile
    bt_tiled = np.ascontiguousarray(
        bt.reshape(NK, P, NCH, CW).transpose(0, 2, 1, 3)
    )
    w2_pack = _build_w2_pack()                            # [128, 384] bf16
    return [
        {
            "at": np.ascontiguousarray(a_t[:, c * MS:(c + 1) * MS]),
            "bt": bt_tiled,
            "w2": w2_pack,
        }
        for c in range(NCORES)
    ]


def _untile_out(res):
    """[NM, NCH, P, CW] per core -> [MS, N]; concatenated across cores."""
    outs = []
    for c in range(NCORES):
        o = res.results[c]["out"]                       # [8, 8, 128, 512]
        outs.append(o.transpose(0, 2, 1, 3).reshape(MS, N))
    return np.concatenate(outs, axis=0).astype(np.float32)


def kernel(A, B):
    A = np.ascontiguousarray(np.asarray(A, dtype=np.float32))
    B = np.ascontiguousarray(np.asarray(B, dtype=np.float32))
    assert A.shape == (M_FULL, N), A.shape
    assert B.shape == (N, N), B.shape

    in_maps = _prep_inputs(A, B)
    nc = _get_program()
    res = run_bass_kernel_spmd(nc, in_maps, core_ids=list(range(NCORES)))
    return _untile_out(res)


# revision 3
# speedup vs baseline: 1.5043x; 1.5043x over previous
"""Trainium2 kernel for out = A @ W2 @ B.T with banded Gaussian W2. (v3)

v3 over v2: DMA count collapsed from ~353 to ~27 per core.
  - bt retiled nu-major on host: one 4MB DMA loads the whole [4096, 512]
    B.T column-block a phase-2 chunk needs (double-buffered).
  - at loaded in 4-block groups (1MB DMAs).
  - out written bf16, one [128, 8*512] tile per nu -> 8 contiguous 1MB
    stores on the Activation HWDGE ring (separate from the SP load ring).
All matmuls bf16 (1 cyc/row, FWL weight loads), PSUM fp32.
"""

import numpy as np
import ml_dtypes

import concourse.bass as bass
import concourse.mybir as mybir
from concourse import bacc
from concourse.bass_utils import run_bass_kernel_spmd
from concourse.tile import TileContext

P = 128          # partition / block size
N = 4096         # inner dims (A cols, B rows/cols)
M_FULL = 8192    # A rows
NCORES = 8
MS = M_FULL // NCORES   # 1024 rows of A per core
NK = N // P      # 32 contraction blocks
NM = MS // P     # 8 m-tiles per core
CW = 512         # output column chunk width (= 1 PSUM bank of fp32)
NCH = N // CW    # 8 chunks
NH = MS // CW    # 2 column-halves of the A.T slab in phase 1
AG = 4           # at blocks per DMA group
NAG = NK // AG   # 8 groups

SIGMA = 8.0
TRUNC_EPS = 1e-10

BF16 = np.dtype(ml_dtypes.bfloat16)

_COMPILED = {}


def _w2_block(dist):
    d = dist.astype(np.float32)
    w1 = np.exp(-(d * d) / np.float32(2.0 * SIGMA * SIGMA)).astype(np.float32)
    w1 = np.where(w1 > np.float32(TRUNC_EPS), w1, np.float32(0.0)).astype(np.float32)
    return (w1 * w1).astype(np.float32)


def _build_w2_pack():
    a = np.arange(P)[:, None]
    b = np.arange(P)[None, :]
    d0 = _w2_block(np.abs(a - b))          # W2[j, j]
    u = _w2_block(np.abs(a - b - P))       # W2[j-1, j]
    l = _w2_block(np.abs(P + a - b))       # W2[j+1, j]
    pack = np.concatenate([d0, u, l], axis=1)  # [128, 384]
    return np.ascontiguousarray(pack.astype(BF16))


def _build_program(reps=1):
    nc = bacc.Bacc("TRN2", target_bir_lowering=False, debug=False)
    f32 = mybir.dt.float32
    bf16 = mybir.dt.bfloat16

    at_dram = nc.dram_tensor("at", [N, MS], bf16, kind="ExternalInput").ap()
    # bt retiled nu-major: bt[nu, k, p, f] = B.T[k*128+p, nu*512+f]
    bt_dram = nc.dram_tensor(
        "bt", [NCH, NK, P, CW], bf16, kind="ExternalInput"
    ).ap()
    w2_dram = nc.dram_tensor("w2", [P, 3 * P], bf16, kind="ExternalInput").ap()
    # out tiled: out[nu, p, m*512+f] = out_slab[m*128+p, nu*512+f], bf16
    out_dram = nc.dram_tensor(
        "out", [NCH, P, NM * CW], bf16, kind="ExternalOutput"
    ).ap()

    with TileContext(nc) as tc:
        with (
            tc.tile_pool(name="const", bufs=1) as const_pool,
            tc.tile_pool(name="atp", bufs=3) as at_pool,
            tc.tile_pool(name="ttp", bufs=1) as tt_pool,
            tc.tile_pool(name="btp", bufs=2) as bt_pool,
            tc.tile_pool(name="obp", bufs=2) as ob_pool,
            tc.tile_pool(name="psp", bufs=8, space="PSUM") as ps_pool,
        ):
            w2_sb = const_pool.tile([P, 3 * P], bf16, tag="w2", name="w2_sb")
            nc.sync.dma_start(w2_sb, w2_dram)
            w2_lhsT = {
                0: w2_sb[:, 0:P],
                -1: w2_sb[:, P:2 * P],
                1: w2_sb[:, 2 * P:3 * P],
            }

            for rep in range(reps):
                # --- phase 1: TT = W2 @ A.T ([4096, 1024] bf16 in SBUF)
                at_groups = [None] * NAG

                def get_at(k, rep=rep):
                    g = k // AG
                    if at_groups[g] is None:
                        at_t = at_pool.tile([P, AG * MS], bf16, tag="at",
                                            name=f"at_sb_{rep}_{g}")
                        # at_t[p, a*MS+f] = at_dram[(g*AG+a)*128 + p, f]
                        nc.sync.dma_start(
                            at_t.rearrange("p (a f) -> p a f", a=AG, f=MS),
                            at_dram[g * AG * P:(g + 1) * AG * P, :]
                            .rearrange("(a p) f -> p a f", a=AG, p=P),
                        )
                        at_groups[g] = at_t
                    kk = k % AG
                    return at_groups[g][:, kk * MS:(kk + 1) * MS]

                tt_tiles = []
                for j in range(NK):
                    tt_t = tt_pool.tile([P, MS], bf16, tag=f"tt{j}",
                                        name=f"tt_sb_{rep}_{j}")
                    dlist = [d for d in (-1, 0, 1) if 0 <= j + d < NK]
                    # prefetch next at group early
                    for h in range(NH):
                        hs = bass.ts(h, CW)
                        ps_t = ps_pool.tile([P, CW], f32, tag="ps",
                                            name=f"ps_t_{rep}_{j}_{h}")
                        for i, d in enumerate(dlist):
                            nc.tensor.matmul(
                                ps_t,
                                lhsT=w2_lhsT[d],
                                rhs=get_at(j + d)[:, hs],
                                start=(i == 0),
                                stop=(i == len(dlist) - 1),
                            )
                        nc.vector.tensor_copy(tt_t[:, hs], ps_t)
                    tt_tiles.append(tt_t)

                # --- phase 2: out = TT.T @ B.T, one 4MB bt column per nu
                bt_cols = [None] * NCH

                def get_bt(nu, rep=rep):
                    if bt_cols[nu] is None:
                        bt_t = bt_pool.tile([P, NK * CW], bf16, tag="bt",
                                            name=f"bt_sb_{rep}_{nu}")
                        # bt_t[p, k*CW+f] = bt_dram[nu, k, p, f]
                        nc.sync.dma_start(
                            bt_t.rearrange("p (k f) -> p k f", k=NK, f=CW),
                            bt_dram[nu].rearrange("k p f -> p k f"),
                        )
                        bt_cols[nu] = bt_t
                    return bt_cols[nu]

                get_bt(0)  # issue first column load before phase-1 finishes

                for nu in range(NCH):
                    bt_t = get_bt(nu)
                    if nu + 1 < NCH:
                        get_bt(nu + 1)  # double-buffer next column
                    ps_o = [
                        ps_pool.tile([P, CW], f32, tag="ps",
                                     name=f"ps_o_{rep}_{nu}_{m}")
                        for m in range(NM)
                    ]
                    for k in range(NK):
                        ks = bass.ts(k, CW)
                        for m in range(NM):
                            nc.tensor.matmul(
                                ps_o[m],
                                lhsT=tt_tiles[k][:, m * P:(m + 1) * P],
                                rhs=bt_t[:, ks],
                                start=(k == 0),
                                stop=(k == NK - 1),
                            )
                    ob_t = ob_pool.tile([P, NM * CW], bf16, tag="ob",
                                        name=f"ob_sb_{rep}_{nu}")
                    for m in range(NM):
                        ms = bass.ts(m, CW)
                        if m % 2 == 0:
                            nc.vector.tensor_copy(ob_t[:, ms], ps_o[m])
                        else:
                            nc.scalar.copy(ob_t[:, ms], ps_o[m])
                    nc.scalar.dma_start(out_dram[nu], ob_t)

    nc.compile()
    return nc


def _get_program():
    if "nc" not in _COMPILED:
        _COMPILED["nc"] = _build_program()
    return _COMPILED["nc"]


def _prep_inputs(A, B):
    a_t = np.ascontiguousarray(A.T).astype(BF16)          # [4096, 8192]
    bt = np.ascontiguousarray(B.T).astype(BF16)           # [4096, 4096]
    # bt[nu, k, p, f] = B.T[k*128+p, nu*512+f]
    bt_tiled = np.ascontiguousarray(
        bt.reshape(NK, P, NCH, CW).transpose(2, 0, 1, 3)
    )
    w2_pack = _build_w2_pack()
    return [
        {
            "at": np.ascontiguousarray(a_t[:, c * MS:(c + 1) * MS]),
            "bt": bt_tiled,
            "w2": w2_pack,
        }
        for c in range(NCORES)
    ]


def _untile_out(res):
    outs = []
    for c in range(NCORES):
        o = np.asarray(res.results[c]["out"])   # [NCH, P, NM*CW] bf16
        o = o.reshape(NCH, P, NM, CW).transpose(2, 1, 0, 3).reshape(MS, N)
        outs.append(o.astype(np.float32))
    return np.concatenate(outs, axis=0)


def kernel(A, B):
    A = np.ascontiguousarray(np.asarray(A, dtype=np.float32))
    B = np.ascontiguousarray(np.asarray(B, dtype=np.float32))
    assert A.shape == (M_FULL, N), A.shape
    assert B.shape == (N, N), B.shape

    in_maps = _prep_inputs(A, B)
    nc = _get_program()
    res = run_bass_kernel_spmd(nc, in_maps, core_ids=list(range(NCORES)))
    return _untile_out(res)
